# revision 14
# baseline (speedup 1.0000x reference)
"""GIN-style 4-layer GNN (N=100000 nodes, E=1600000 edges, 128 features) on
8 Trainium2 NeuronCores.

Sharding: nodes are partitioned across the 8 cores by destination id (12500
nodes/core).  Each layer:
  1. every core gathers the messages h[src] for its in-edges with
     gpsimd.dma_gather (bf16 rows, 256B each) from a replicated node table in
     its local DRAM; edges are pre-sorted host-side by (dst block of 128,
     src chunk of 25000) and padded to 128-edge subtiles so indices fit the
     gather's int16 window,
  2. scatter-add is done on the TensorEngine: for each 128-edge subtile an
     edge->dst-slot one-hot (built on the VectorEngine via is_equal against an
     iota row) is multiplied against the gathered messages, accumulating the
     feature-major aggregate in PSUM,
  3. z = h + agg, then the (BN-folded) Linear-ReLU-Linear MLP runs in f32 with
     per-partition bias+ReLU on the ScalarEngine,
  4. the new h is PE-transposed to node-major bf16 rows and republished to all
     cores with an AllGather so the next layer can gather from it.
"""

import sys

sys.path.insert(0, "/opt/trn_rl_repo")

import numpy as np
import ml_dtypes

import concourse.bacc as bacc
import concourse.mybir as mybir
import concourse.tile as tile
from concourse.tile_rust import add_dep_helper
from concourse.bass_utils import run_bass_kernel_spmd

N = 100000
E = 1600000
F = 128
NLAYERS = 4
NCORES = 8
NPC = N // NCORES            # 12500 nodes per core
NBLK = (NPC + 127) // 128    # 98 dst blocks per core (last has 84 nodes)
CHW = 25000                  # gather-window rows (int16 index limit is 32767)
NCH = N // CHW               # 4 chunks
BG = 4                       # dst blocks per node-group (512 nodes = 1 PSUM bank)
NBG = (NBLK + BG - 1) // BG  # 25 node groups
BN_EPS = 1e-5
PAD_DSTL = 200.0             # one-hot sentinel (>=128 -> all-zero row)

bf16 = ml_dtypes.bfloat16


def _prep(x, edge_index, params):
    """Host-side preprocessing: edge bucketing/padding, weight folding."""
    src = edge_index[0].astype(np.int64)
    dst = edge_index[1].astype(np.int64)
    core = dst // NPC
    dln = dst % NPC
    blk = dln // 128
    dstl = (dln % 128).astype(np.float32)
    ch = src // CHW
    srcl = (src % CHW).astype(np.int16)

    gkey = (core * NBLK + blk) * NCH + ch
    ng = NCORES * NBLK * NCH
    cnt = np.bincount(gkey, minlength=ng).reshape(NCORES, NBLK, NCH)
    # uniform (over cores) subtile counts per (block, chunk)
    K = -(-cnt.max(axis=0) // 128)  # [NBLK, NCH] ceil
    # subtile prefix in emission order: (bg, c, b-in-bg)
    P = np.zeros((NBLK, NCH), np.int64)
    run = 0
    for bg in range(NBG):
        blo, bhi = bg * BG, min((bg + 1) * BG, NBLK)
        for c in range(NCH):
            for b in range(blo, bhi):
                P[b, c] = run
                run += K[b, c]
    nsubt = int(run)

    # slot position for every edge: P[b,c]*128 + rank within (core,b,c) group
    order = np.argsort(gkey, kind="stable")
    gks = gkey[order]
    gstart = np.zeros(ng + 1, np.int64)
    np.cumsum(np.bincount(gks, minlength=ng), out=gstart[1:])
    rank = np.arange(E) - gstart[gks]
    pos = P[blk[order], ch[order]] * 128 + rank

    idx_all = np.zeros((NCORES, nsubt * 128), np.int16)
    dstl_all = np.full((NCORES, nsubt * 128), PAD_DSTL, np.float32)
    idx_all[core[order], pos] = srcl[order]
    dstl_all[core[order], pos] = dstl[order]

    # wrap for dma_gather: slot j -> partition j%16, col j//16; replicate x8
    idx_w = np.ascontiguousarray(
        np.tile(idx_all.reshape(NCORES, nsubt * 8, 16).transpose(0, 2, 1), (1, 8, 1))
    )  # [NCORES, 128, nsubt*8]
    dstl_w = np.ascontiguousarray(
        dstl_all.reshape(NCORES, nsubt, 128).transpose(0, 2, 1)
    ).astype(bf16)  # [NCORES, 128, nsubt]

    # BN-folded MLP weights: layer l uses wp[:, (2l+m)*128:...], bp[:, 2l+m]
    wp = np.zeros((128, 2 * NLAYERS * 128), np.float32)
    bp = np.zeros((128, 2 * NLAYERS), np.float32)
    inv = np.float32(1.0 / np.sqrt(np.float32(1.0 + BN_EPS)))
    for l, (W1, b1, g1, be1, W2, b2, g2, be2) in enumerate(params):
        s1 = np.asarray(g1, np.float32) * inv
        s2 = np.asarray(g2, np.float32) * inv
        wp[:, (2 * l) * 128 : (2 * l + 1) * 128] = np.asarray(W1, np.float32) * s1
        wp[:, (2 * l + 1) * 128 : (2 * l + 2) * 128] = np.asarray(W2, np.float32) * s2
        bp[:, 2 * l] = np.asarray(b1, np.float32) * s1 + np.asarray(be1, np.float32)
        bp[:, 2 * l + 1] = np.asarray(b2, np.float32) * s2 + np.asarray(be2, np.float32)

    x = np.asarray(x, np.float32)
    x_bf = x.astype(bf16)
    iota = np.ascontiguousarray(
        np.broadcast_to(np.arange(128, dtype=np.float32), (128, 128))
    ).astype(bf16)
    ident = np.eye(128, dtype=np.float32)

    in_maps = []
    for c in range(NCORES):
        in_maps.append(
            {
                "xT": np.ascontiguousarray(x[c * NPC : (c + 1) * NPC].T),
                "x_bf": x_bf,
                "idx": idx_w[c],
                "dstl": dstl_w[c],
                "wp": wp,
                "bp": bp,
                "iota": iota,
                "ident": ident,
            }
        )
    return in_maps, K, nsubt


def _build(K, nsubt):
    dt = mybir.dt
    nc = bacc.Bacc("TRN2", num_devices=NCORES)

    xT_in = nc.dram_tensor("xT", [128, NPC], dt.float32, kind="ExternalInput")
    x_bf_in = nc.dram_tensor("x_bf", [N, F], dt.bfloat16, kind="ExternalInput")
    idx_in = nc.dram_tensor("idx", [128, nsubt * 8], dt.int16, kind="ExternalInput")
    dstl_in = nc.dram_tensor("dstl", [128, nsubt], dt.bfloat16, kind="ExternalInput")
    wp_in = nc.dram_tensor("wp", [128, 2 * NLAYERS * 128], dt.float32,
                           kind="ExternalInput")
    bp_in = nc.dram_tensor("bp", [128, 2 * NLAYERS], dt.float32,
                           kind="ExternalInput")
    iota_in = nc.dram_tensor("iota", [128, 128], dt.bfloat16, kind="ExternalInput")
    ident_in = nc.dram_tensor("ident", [128, 128], dt.float32, kind="ExternalInput")
    out_t = nc.dram_tensor("out", [NPC, F], dt.float32, kind="ExternalOutput")

    # widths of each node-group / block
    def bwidth(b):
        return min(128, NPC - b * 128)

    def gwidth(bg):
        return min(512, NPC - bg * 512)

    with tile.TileContext(nc) as tc:
        with (
            tc.tile_pool(name="const", bufs=1) as constp,
            tc.tile_pool(name="hbuf", bufs=2) as hbufp,
            tc.tile_pool(name="gath", bufs=6) as gathp,
            tc.tile_pool(name="oh", bufs=5) as ohp,
            tc.tile_pool(name="idxs", bufs=4) as idxp,
            tc.tile_pool(name="work", bufs=2) as workp,
            tc.tile_pool(name="stage", bufs=2) as stagep,
            tc.tile_pool(name="psA", bufs=2, space="PSUM") as psA,
            tc.tile_pool(name="psM", bufs=2, space="PSUM") as psM,
            tc.tile_pool(name="psT", bufs=2, space="PSUM") as psT,
            tc.tile_pool(name="dram", bufs=1, space="DRAM") as dram,
        ):
            # constants
            dstl_t = constp.tile([128, nsubt], dt.bfloat16, tag="dstl")
            nc.sync.dma_start(dstl_t[:], dstl_in[:])
            wp_t = constp.tile([128, 2 * NLAYERS * 128], dt.float32, tag="wp")
            nc.sync.dma_start(wp_t[:], wp_in[:])
            bp_t = constp.tile([128, 2 * NLAYERS], dt.float32, tag="bp")
            nc.sync.dma_start(bp_t[:], bp_in[:])
            iota_t = constp.tile([128, 128], dt.bfloat16, tag="iota")
            nc.sync.dma_start(iota_t[:], iota_in[:])
            ident_t = constp.tile([128, 128], dt.float32, tag="ident")
            nc.sync.dma_start(ident_t[:], ident_in[:])

            # DRAM: two alternating replicated node tables + AG input shards
            tabs = [
                dram.tile([N, F], dt.bfloat16, addr_space="Shared",
                          tag=f"tab{i}", name=f"tab{i}")
                for i in range(NLAYERS - 1)
            ]
            shards = [
                dram.tile([NPC, F], dt.bfloat16, tag=f"sh{i}", name=f"sh{i}")
                for i in range(NLAYERS - 1)
            ]

            # h^T (feature-major, f32) for the current layer
            h_old = hbufp.tile([128, NPC], dt.float32, tag="hT")
            nc.sync.dma_start(h_old[:], xT_in[:])

            kmax = max(
                int(K[bg * BG : min((bg + 1) * BG, NBLK), c].sum())
                for bg in range(NBG)
                for c in range(NCH)
            )

            # the one-hot TTs and idx-stream DMAs depend only on constants, so
            # without explicit ordering the scheduler hands their ring slots
            # to far-future instances and wedges; pace them on the gathers.
            gather_insts = []

            for l in range(NLAYERS):
                table = x_bf_in if l == 0 else tabs[l - 1]
                h_new = hbufp.tile([128, NPC], dt.float32, tag="hT")

                for bg in range(NBG):
                    blo, bhi = bg * BG, min((bg + 1) * BG, NBLK)
                    gw = gwidth(bg)

                    # gathers + one-hots per source chunk
                    gts, ohs = [], []
                    for c in range(NCH):
                        S = int(K[blo:bhi, c].sum())
                        if S == 0:
                            gts.append(None)
                            ohs.append(None)
                            continue
                        # subtile prefix of this (bg, c) group
                        p0 = min(
                            _P_LOOKUP[(b, c)] for b in range(blo, bhi) if K[b, c] > 0
                        )
                        nidx = S * 128
                        it = idxp.tile([128, kmax * 8], dt.int16, tag="idx")
                        idma = nc.sync.dma_start(
                            it[:, : S * 8], idx_in[:, p0 * 8 : (p0 + S) * 8]
                        )
                        if len(gather_insts) >= 3:
                            add_dep_helper(
                                idma.ins, gather_insts[-3].ins, sync=False,
                                reason="pace idx stream on gather pipeline",
                            )
                        gt = gathp.tile([128, kmax, F], dt.bfloat16, tag="g")
                        gi = nc.gpsimd.dma_gather(
                            gt[:, :S, :],
                            table[c * CHW : (c + 1) * CHW, :],
                            it[:, : S * 8],
                            nidx,
                            nidx,
                            F,
                            single_packet=False,
                        )
                        gather_insts.append(gi)
                        oh = ohp.tile([128, kmax, 128], dt.bfloat16, tag="oh")
                        ohi = nc.vector.tensor_tensor(
                            oh[:, :S, :],
                            iota_t[:, :].unsqueeze(1).broadcast_to([128, S, 128]),
                            dstl_t[:, p0 : p0 + S]
                            .unsqueeze(2)
                            .broadcast_to([128, S, 128]),
                            mybir.AluOpType.is_equal,
                        )
                        add_dep_helper(
                            ohi.ins, gi.ins, sync=False,
                            reason="pace one-hot gen on gather pipeline",
                        )
                        gts.append(gt)
                        ohs.append(oh)

                    # one-hot matmuls accumulate agg^T[f, dst] in PSUM.
                    # block-major: each 128-dst region's accumulation group is
                    # contiguous — interleaving start/stop groups within one
                    # PSUM tile produced wrong results on HW.
                    agg = psA.tile([128, 512], dt.float32, tag="agg")
                    for bi, b in enumerate(range(blo, bhi)):
                        cs = [cc for cc in range(NCH) if K[b, cc] > 0]
                        for c in cs:
                            p0g = min(
                                _P_LOOKUP[(bb, c)]
                                for bb in range(blo, bhi)
                                if K[bb, c] > 0
                            )
                            sb = _P_LOOKUP[(b, c)] - p0g
                            for s in range(int(K[b, c])):
                                nc.tensor.matmul(
                                    agg[:, bi * 128 : bi * 128 + 128],
                                    gts[c][:, sb + s, :],
                                    ohs[c][:, sb + s, :],
                                    start=(c == cs[0] and s == 0),
                                    stop=(c == cs[-1] and s == int(K[b, c]) - 1),
                                )

                    # z = agg + h_old  (blocks with no edges at all: z = h_old)
                    z = workp.tile([128, 512], dt.float32, tag="z")
                    for bi, b in enumerate(range(blo, bhi)):
                        wb = bwidth(b)
                        zsl = z[:, bi * 128 : bi * 128 + wb]
                        hsl = h_old[:, b * 128 : b * 128 + wb]
                        if any(K[b, c] > 0 for c in range(NCH)):
                            nc.vector.tensor_tensor(
                                zsl,
                                agg[:, bi * 128 : bi * 128 + wb],
                                hsl,
                                mybir.AluOpType.add,
                            )
                        else:
                            nc.vector.tensor_copy(zsl, hsl)

                    # MLP: relu(z @ W1 + b1) @ W2 + b2 (BN folded in)
                    ps1 = psM.tile([128, 512], dt.float32, tag="mlp")
                    nc.tensor.matmul(
                        ps1[:, :gw],
                        wp_t[:, (2 * l) * 128 : (2 * l + 1) * 128],
                        z[:, :gw],
                        start=True,
                        stop=True,
                    )
                    h1 = workp.tile([128, 512], dt.float32, tag="h1")
                    nc.scalar.activation(
                        h1[:, :gw],
                        ps1[:, :gw],
                        mybir.ActivationFunctionType.Relu,
                        bias=bp_t[:, 2 * l : 2 * l + 1],
                        scale=1.0,
                    )
                    ps2 = psM.tile([128, 512], dt.float32, tag="mlp")
                    nc.tensor.matmul(
                        ps2[:, :gw],
                        wp_t[:, (2 * l + 1) * 128 : (2 * l + 2) * 128],
                        h1[:, :gw],
                        start=True,
                        stop=True,
                    )
                    nc.scalar.activation(
                        h_new[:, bg * 512 : bg * 512 + gw],
                        ps2[:, :gw],
                        mybir.ActivationFunctionType.Relu
                        if l < NLAYERS - 1
                        else mybir.ActivationFunctionType.Identity,
                        bias=bp_t[:, 2 * l + 1 : 2 * l + 2],
                        scale=1.0,
                    )

                    # transpose to node-major and write shard rows
                    if l < NLAYERS - 1:
                        st = stagep.tile([128, BG, F], dt.bfloat16, tag="stb")
                    else:
                        st = stagep.tile([128, BG, F], dt.float32, tag="stf")
                    for bi, b in enumerate(range(blo, bhi)):
                        wb = bwidth(b)
                        pt = psT.tile([128, 128], dt.float32, tag="tr")
                        nc.tensor.transpose(
                            pt[:wb, :],
                            h_new[:, b * 128 : b * 128 + wb],
                            ident_t[:],
                        )
                        nc.scalar.activation(
                            st[:wb, bi, :],
                            pt[:wb, :],
                            mybir.ActivationFunctionType.Copy,
                        )
                    dst_t = shards[l] if l < NLAYERS - 1 else out_t
                    for bi, b in enumerate(range(blo, bhi)):
                        wb = bwidth(b)
                        nc.sync.dma_start(
                            dst_t[b * 128 : b * 128 + wb, :],
                            st[:wb, bi, :],
                        )

                if l < NLAYERS - 1:
                    nc.gpsimd.collective_compute(
                        "AllGather",
                        mybir.AluOpType.bypass,
                        replica_groups=[list(range(NCORES))],
                        ins=[shards[l].opt()],
                        outs=[tabs[l].opt()],
                    )
                h_old = h_new

    nc.compile()
    return nc


_P_LOOKUP = {}


def _set_p_lookup(K):
    _P_LOOKUP.clear()
    run = 0
    for bg in range(NBG):
        blo, bhi = bg * BG, min((bg + 1) * BG, NBLK)
        for c in range(NCH):
            for b in range(blo, bhi):
                _P_LOOKUP[(b, c)] = run
                run += int(K[b, c])


def kernel(x, edge_index, params):
    in_maps, K, nsubt = _prep(x, edge_index, params)
    _set_p_lookup(K)
    nc = _build(K, nsubt)
    res = run_bass_kernel_spmd(nc, in_maps, core_ids=list(range(NCORES)))
    return np.concatenate([res.results[c]["out"] for c in range(NCORES)], axis=0)


def run_traced(x, edge_index, params):
    """For test.py: returns (output, BassKernelResults)."""
    in_maps, K, nsubt = _prep(x, edge_index, params)
    _set_p_lookup(K)
    nc = _build(K, nsubt)
    res = run_bass_kernel_spmd(
        nc, in_maps, core_ids=list(range(NCORES)), trace=True
    )
    out = np.concatenate([res.results[c]["out"] for c in range(NCORES)], axis=0)
    return out, res


# revision 15
# speedup vs baseline: 1.6825x; 1.6825x over previous
"""GIN-style 4-layer GNN (N=100000 nodes, E=1600000 edges, 128 features) on
8 Trainium2 NeuronCores.

Sharding: nodes are partitioned across the 8 cores by destination id (12500
nodes/core).  Each layer:
  1. every core gathers the messages h[src] for its in-edges with
     gpsimd.dma_gather (bf16 rows, 256B each) from a replicated node table in
     its local DRAM; edges are pre-sorted host-side by (dst block of 128,
     src chunk of 25000) and padded to 128-edge subtiles so indices fit the
     gather's int16 window,
  2. scatter-add is done on the TensorEngine: for each 128-edge subtile an
     edge->dst-slot one-hot (built on the VectorEngine via is_equal against an
     iota row) is multiplied against the gathered messages, accumulating the
     feature-major aggregate in PSUM,
  3. z = h + agg, then the (BN-folded) Linear-ReLU-Linear MLP runs in f32 with
     per-partition bias+ReLU on the ScalarEngine,
  4. the new h is PE-transposed to node-major bf16 rows and republished to all
     cores with an AllGather so the next layer can gather from it.
"""

import sys

sys.path.insert(0, "/opt/trn_rl_repo")

import numpy as np
import ml_dtypes

import concourse.bacc as bacc
import concourse.mybir as mybir
import concourse.tile as tile
from concourse.tile_rust import add_dep_helper
from concourse.bass_utils import run_bass_kernel_spmd

N = 100000
E = 1600000
F = 128
NLAYERS = 4
NCORES = 8
NPC = N // NCORES            # 12500 nodes per core
NBLK = (NPC + 127) // 128    # 98 dst blocks per core (last has 84 nodes)
CHW = 25000                  # gather-window rows (int16 index limit is 32767)
NCH = N // CHW               # 4 chunks
BG = 4                       # dst blocks per node-group (512 nodes = 1 PSUM bank)
NBG = (NBLK + BG - 1) // BG  # 25 node groups
BN_EPS = 1e-5
PAD_DSTL = 200.0             # one-hot sentinel (>=128 -> all-zero row)

bf16 = ml_dtypes.bfloat16


def _prep(x, edge_index, params):
    """Host-side preprocessing: edge bucketing/padding, weight folding."""
    src = edge_index[0].astype(np.int64)
    dst = edge_index[1].astype(np.int64)
    core = dst // NPC
    dln = dst % NPC
    blk = dln // 128
    dstl = (dln % 128).astype(np.float32)
    ch = src // CHW
    srcl = (src % CHW).astype(np.int16)

    gkey = (core * NBLK + blk) * NCH + ch
    ng = NCORES * NBLK * NCH
    cnt = np.bincount(gkey, minlength=ng).reshape(NCORES, NBLK, NCH)
    # uniform (over cores) subtile counts per (block, chunk)
    K = -(-cnt.max(axis=0) // 128)  # [NBLK, NCH] ceil
    # subtile prefix in emission order: (bg, c, b-in-bg)
    P = np.zeros((NBLK, NCH), np.int64)
    run = 0
    for bg in range(NBG):
        blo, bhi = bg * BG, min((bg + 1) * BG, NBLK)
        for c in range(NCH):
            for b in range(blo, bhi):
                P[b, c] = run
                run += K[b, c]
    nsubt = int(run)

    # slot position for every edge: P[b,c]*128 + rank within (core,b,c) group
    order = np.argsort(gkey, kind="stable")
    gks = gkey[order]
    gstart = np.zeros(ng + 1, np.int64)
    np.cumsum(np.bincount(gks, minlength=ng), out=gstart[1:])
    rank = np.arange(E) - gstart[gks]
    pos = P[blk[order], ch[order]] * 128 + rank

    idx_all = np.zeros((NCORES, nsubt * 128), np.int16)
    dstl_all = np.full((NCORES, nsubt * 128), PAD_DSTL, np.float32)
    idx_all[core[order], pos] = srcl[order]
    dstl_all[core[order], pos] = dstl[order]

    # wrap for dma_gather: slot j -> partition j%16, col j//16; replicate x8
    idx_w = np.ascontiguousarray(
        np.tile(idx_all.reshape(NCORES, nsubt * 8, 16).transpose(0, 2, 1), (1, 8, 1))
    )  # [NCORES, 128, nsubt*8]
    dstl_w = np.ascontiguousarray(
        dstl_all.reshape(NCORES, nsubt, 128).transpose(0, 2, 1)
    ).astype(bf16)  # [NCORES, 128, nsubt]

    # BN-folded MLP weights: layer l uses wp[:, (2l+m)*128:...], bp[:, 2l+m]
    wp = np.zeros((128, 2 * NLAYERS * 128), np.float32)
    bp = np.zeros((128, 2 * NLAYERS), np.float32)
    inv = np.float32(1.0 / np.sqrt(np.float32(1.0 + BN_EPS)))
    for l, (W1, b1, g1, be1, W2, b2, g2, be2) in enumerate(params):
        s1 = np.asarray(g1, np.float32) * inv
        s2 = np.asarray(g2, np.float32) * inv
        wp[:, (2 * l) * 128 : (2 * l + 1) * 128] = np.asarray(W1, np.float32) * s1
        wp[:, (2 * l + 1) * 128 : (2 * l + 2) * 128] = np.asarray(W2, np.float32) * s2
        bp[:, 2 * l] = np.asarray(b1, np.float32) * s1 + np.asarray(be1, np.float32)
        bp[:, 2 * l + 1] = np.asarray(b2, np.float32) * s2 + np.asarray(be2, np.float32)

    x = np.asarray(x, np.float32)
    x_bf = x.astype(bf16)
    iota = np.ascontiguousarray(
        np.broadcast_to(np.arange(128, dtype=np.float32), (128, 128))
    ).astype(bf16)
    ident = np.eye(128, dtype=np.float32)

    in_maps = []
    for c in range(NCORES):
        in_maps.append(
            {
                "xT": np.ascontiguousarray(x[c * NPC : (c + 1) * NPC].T),
                "x_bf": x_bf,
                "idx": idx_w[c],
                "dstl": dstl_w[c],
                "wp": wp,
                "bp": bp,
                "iota": iota,
                "ident": ident,
            }
        )
    return in_maps, K, nsubt


def _build(K, nsubt):
    dt = mybir.dt
    nc = bacc.Bacc("TRN2", num_devices=NCORES, num_swdge_queues=4)

    xT_in = nc.dram_tensor("xT", [128, NPC], dt.float32, kind="ExternalInput")
    x_bf_in = nc.dram_tensor("x_bf", [N, F], dt.bfloat16, kind="ExternalInput")
    idx_in = nc.dram_tensor("idx", [128, nsubt * 8], dt.int16, kind="ExternalInput")
    dstl_in = nc.dram_tensor("dstl", [128, nsubt], dt.bfloat16, kind="ExternalInput")
    wp_in = nc.dram_tensor("wp", [128, 2 * NLAYERS * 128], dt.float32,
                           kind="ExternalInput")
    bp_in = nc.dram_tensor("bp", [128, 2 * NLAYERS], dt.float32,
                           kind="ExternalInput")
    iota_in = nc.dram_tensor("iota", [128, 128], dt.bfloat16, kind="ExternalInput")
    ident_in = nc.dram_tensor("ident", [128, 128], dt.float32, kind="ExternalInput")
    out_t = nc.dram_tensor("out", [NPC, F], dt.float32, kind="ExternalOutput")

    # widths of each node-group / block
    def bwidth(b):
        return min(128, NPC - b * 128)

    def gwidth(bg):
        return min(512, NPC - bg * 512)

    with tile.TileContext(nc) as tc:
        with (
            tc.tile_pool(name="const", bufs=1) as constp,
            tc.tile_pool(name="hbuf", bufs=2) as hbufp,
            tc.tile_pool(name="gath", bufs=6) as gathp,
            tc.tile_pool(name="oh", bufs=5) as ohp,
            tc.tile_pool(name="idxs", bufs=4) as idxp,
            tc.tile_pool(name="work", bufs=2) as workp,
            tc.tile_pool(name="stage", bufs=2) as stagep,
            tc.tile_pool(name="psA", bufs=2, space="PSUM") as psA,
            tc.tile_pool(name="psM", bufs=2, space="PSUM") as psM,
            tc.tile_pool(name="psT", bufs=2, space="PSUM") as psT,
            tc.tile_pool(name="dram", bufs=1, space="DRAM") as dram,
        ):
            # constants
            dstl_t = constp.tile([128, nsubt], dt.bfloat16, tag="dstl")
            nc.sync.dma_start(dstl_t[:], dstl_in[:])
            wp_t = constp.tile([128, 2 * NLAYERS * 128], dt.float32, tag="wp")
            nc.sync.dma_start(wp_t[:], wp_in[:])
            bp_t = constp.tile([128, 2 * NLAYERS], dt.float32, tag="bp")
            nc.sync.dma_start(bp_t[:], bp_in[:])
            iota_t = constp.tile([128, 128], dt.bfloat16, tag="iota")
            nc.sync.dma_start(iota_t[:], iota_in[:])
            ident_t = constp.tile([128, 128], dt.float32, tag="ident")
            nc.sync.dma_start(ident_t[:], ident_in[:])

            # DRAM: two alternating replicated node tables + AG input shards
            tabs = [
                dram.tile([N, F], dt.bfloat16, addr_space="Shared",
                          tag=f"tab{i}", name=f"tab{i}")
                for i in range(NLAYERS - 1)
            ]
            shards = [
                dram.tile([NPC, F], dt.bfloat16, tag=f"sh{i}", name=f"sh{i}")
                for i in range(NLAYERS - 1)
            ]

            # h^T (feature-major, f32) for the current layer
            h_old = hbufp.tile([128, NPC], dt.float32, tag="hT")
            nc.sync.dma_start(h_old[:], xT_in[:])

            kmax = max(
                int(K[bg * BG : min((bg + 1) * BG, NBLK), c].sum())
                for bg in range(NBG)
                for c in range(NCH)
            )

            # the one-hot TTs and idx-stream DMAs depend only on constants, so
            # without explicit ordering the scheduler hands their ring slots
            # to far-future instances and wedges; pace them on the gathers.
            gather_insts = []

            for l in range(NLAYERS):
                table = x_bf_in if l == 0 else tabs[l - 1]
                h_new = hbufp.tile([128, NPC], dt.float32, tag="hT")

                for bg in range(NBG):
                    blo, bhi = bg * BG, min((bg + 1) * BG, NBLK)
                    gw = gwidth(bg)

                    # gathers + one-hots per source chunk
                    gts, ohs = [], []
                    for c in range(NCH):
                        S = int(K[blo:bhi, c].sum())
                        if S == 0:
                            gts.append(None)
                            ohs.append(None)
                            continue
                        # subtile prefix of this (bg, c) group
                        p0 = min(
                            _P_LOOKUP[(b, c)] for b in range(blo, bhi) if K[b, c] > 0
                        )
                        nidx = S * 128
                        it = idxp.tile([128, kmax * 8], dt.int16, tag="idx")
                        idma = nc.sync.dma_start(
                            it[:, : S * 8], idx_in[:, p0 * 8 : (p0 + S) * 8]
                        )
                        if len(gather_insts) >= 3:
                            add_dep_helper(
                                idma.ins, gather_insts[-3].ins, sync=False,
                                reason="pace idx stream on gather pipeline",
                            )
                        gt = gathp.tile([128, kmax, F], dt.bfloat16, tag="g")
                        gi = nc.gpsimd.dma_gather(
                            gt[:, :S, :],
                            table[c * CHW : (c + 1) * CHW, :],
                            it[:, : S * 8],
                            nidx,
                            nidx,
                            F,
                            single_packet=False,
                            queue_num=c,
                        )
                        gather_insts.append(gi)
                        oh = ohp.tile([128, kmax, 128], dt.bfloat16, tag="oh")
                        ohi = nc.vector.tensor_tensor(
                            oh[:, :S, :],
                            iota_t[:, :].unsqueeze(1).broadcast_to([128, S, 128]),
                            dstl_t[:, p0 : p0 + S]
                            .unsqueeze(2)
                            .broadcast_to([128, S, 128]),
                            mybir.AluOpType.is_equal,
                        )
                        add_dep_helper(
                            ohi.ins, gi.ins, sync=False,
                            reason="pace one-hot gen on gather pipeline",
                        )
                        gts.append(gt)
                        ohs.append(oh)

                    # one-hot matmuls accumulate agg^T[f, dst] in PSUM.
                    # block-major: each 128-dst region's accumulation group is
                    # contiguous — interleaving start/stop groups within one
                    # PSUM tile produced wrong results on HW.
                    agg = psA.tile([128, 512], dt.float32, tag="agg")
                    for bi, b in enumerate(range(blo, bhi)):
                        cs = [cc for cc in range(NCH) if K[b, cc] > 0]
                        for c in cs:
                            p0g = min(
                                _P_LOOKUP[(bb, c)]
                                for bb in range(blo, bhi)
                                if K[bb, c] > 0
                            )
                            sb = _P_LOOKUP[(b, c)] - p0g
                            for s in range(int(K[b, c])):
                                nc.tensor.matmul(
                                    agg[:, bi * 128 : bi * 128 + 128],
                                    gts[c][:, sb + s, :],
                                    ohs[c][:, sb + s, :],
                                    start=(c == cs[0] and s == 0),
                                    stop=(c == cs[-1] and s == int(K[b, c]) - 1),
                                )

                    # z = agg + h_old  (blocks with no edges at all: z = h_old)
                    z = workp.tile([128, 512], dt.float32, tag="z")
                    for bi, b in enumerate(range(blo, bhi)):
                        wb = bwidth(b)
                        zsl = z[:, bi * 128 : bi * 128 + wb]
                        hsl = h_old[:, b * 128 : b * 128 + wb]
                        if any(K[b, c] > 0 for c in range(NCH)):
                            nc.vector.tensor_tensor(
                                zsl,
                                agg[:, bi * 128 : bi * 128 + wb],
                                hsl,
                                mybir.AluOpType.add,
                            )
                        else:
                            nc.vector.tensor_copy(zsl, hsl)

                    # MLP: relu(z @ W1 + b1) @ W2 + b2 (BN folded in)
                    ps1 = psM.tile([128, 512], dt.float32, tag="mlp")
                    nc.tensor.matmul(
                        ps1[:, :gw],
                        wp_t[:, (2 * l) * 128 : (2 * l + 1) * 128],
                        z[:, :gw],
                        start=True,
                        stop=True,
                    )
                    h1 = workp.tile([128, 512], dt.float32, tag="h1")
                    nc.scalar.activation(
                        h1[:, :gw],
                        ps1[:, :gw],
                        mybir.ActivationFunctionType.Relu,
                        bias=bp_t[:, 2 * l : 2 * l + 1],
                        scale=1.0,
                    )
                    ps2 = psM.tile([128, 512], dt.float32, tag="mlp")
                    nc.tensor.matmul(
                        ps2[:, :gw],
                        wp_t[:, (2 * l + 1) * 128 : (2 * l + 2) * 128],
                        h1[:, :gw],
                        start=True,
                        stop=True,
                    )
                    nc.scalar.activation(
                        h_new[:, bg * 512 : bg * 512 + gw],
                        ps2[:, :gw],
                        mybir.ActivationFunctionType.Relu
                        if l < NLAYERS - 1
                        else mybir.ActivationFunctionType.Identity,
                        bias=bp_t[:, 2 * l + 1 : 2 * l + 2],
                        scale=1.0,
                    )

                    # transpose to node-major and write shard rows
                    if l < NLAYERS - 1:
                        st = stagep.tile([128, BG, F], dt.bfloat16, tag="stb")
                    else:
                        st = stagep.tile([128, BG, F], dt.float32, tag="stf")
                    for bi, b in enumerate(range(blo, bhi)):
                        wb = bwidth(b)
                        pt = psT.tile([128, 128], dt.float32, tag="tr")
                        nc.tensor.transpose(
                            pt[:wb, :],
                            h_new[:, b * 128 : b * 128 + wb],
                            ident_t[:],
                        )
                        nc.scalar.activation(
                            st[:wb, bi, :],
                            pt[:wb, :],
                            mybir.ActivationFunctionType.Copy,
                        )
                    dst_t = shards[l] if l < NLAYERS - 1 else out_t
                    for bi, b in enumerate(range(blo, bhi)):
                        wb = bwidth(b)
                        nc.sync.dma_start(
                            dst_t[b * 128 : b * 128 + wb, :],
                            st[:wb, bi, :],
                        )

                if l < NLAYERS - 1:
                    nc.gpsimd.collective_compute(
                        "AllGather",
                        mybir.AluOpType.bypass,
                        replica_groups=[list(range(NCORES))],
                        ins=[shards[l].opt()],
                        outs=[tabs[l].opt()],
                    )
                h_old = h_new

    nc.compile()
    return nc


_P_LOOKUP = {}


def _set_p_lookup(K):
    _P_LOOKUP.clear()
    run = 0
    for bg in range(NBG):
        blo, bhi = bg * BG, min((bg + 1) * BG, NBLK)
        for c in range(NCH):
            for b in range(blo, bhi):
                _P_LOOKUP[(b, c)] = run
                run += int(K[b, c])


def kernel(x, edge_index, params):
    in_maps, K, nsubt = _prep(x, edge_index, params)
    _set_p_lookup(K)
    nc = _build(K, nsubt)
    res = run_bass_kernel_spmd(nc, in_maps, core_ids=list(range(NCORES)))
    return np.concatenate([res.results[c]["out"] for c in range(NCORES)], axis=0)


def run_traced(x, edge_index, params):
    """For test.py: returns (output, BassKernelResults)."""
    in_maps, K, nsubt = _prep(x, edge_index, params)
    _set_p_lookup(K)
    nc = _build(K, nsubt)
    res = run_bass_kernel_spmd(
        nc, in_maps, core_ids=list(range(NCORES)), trace=True
    )
    out = np.concatenate([res.results[c]["out"] for c in range(NCORES)], axis=0)
    return out, res
